# revision 1
# baseline (speedup 1.0000x reference)
"""Trainium2 Bass kernel for the MLPConstructor2 adjacency problem.

Computes, per batch b (one NeuronCore each, 8-way data parallel over B):
    adj[i, j] = tanh(relu(x1_i @ w1 + x2_j @ w2 + b))
for the four (spatial/temporal) quadrants of a (2560, 2560) output.

The output is an outer broadcast-sum of per-row and per-column scalar
vectors, so the kernel is HBM-write bound (26.2 MB/core). Design:

- x is staged twice, in (t p) layout for the row-side stats (so each
  128-row output tile's biases land on partitions directly) and in (p t)
  layout for the col-side stats (so the stat tile streams out to a DRAM
  scratch contiguously in row order -- no transpose anywhere).
- All eight dot-product vectors are mul(+step-0-broadcast weight)/reduce
  on VectorE; quadrant biases are folded into the col-side stats.
- The column vectors are replicated across partitions with a single
  partition-step-0 DMA broadcast-load of the scratch: pure DMA broadcast.
  No PE, no PSUM (fp32 PE matmuls are 4 cycles/row and cold-clocked).
- Main loop per 128-row output tile: 2 ScalarE tanh activations
  (per-quadrant per-partition row bias), 1 VectorE relu in place, one
  1.31 MB contiguous store, alternating Sync (HWDGE) / GpSimd (SWDGE)
  rings so two DMA queues drain in parallel.
"""

import numpy as np
from contextlib import ExitStack

import concourse.bacc as bacc
import concourse.mybir as mybir
import concourse.tile as tile
from concourse.bass_utils import run_bass_kernel_spmd

B, N, T, D = 8, 2048, 512, 32
W = N + T            # 2560
NT, TT = N // 128, T // 128   # 16, 4 row-tiles
F32 = mybir.dt.float32
QUADS = ("ss", "st", "ts", "tt")


def _emit(tc, sp, tm, ws, scr, adj):
    nc = tc.nc
    AF = mybir.ActivationFunctionType
    OP = mybir.AluOpType
    with ExitStack() as ctx:
        const = ctx.enter_context(tc.tile_pool(name="const", bufs=1))
        outp = ctx.enter_context(tc.tile_pool(name="outp", bufs=8))

        # ---- stage inputs, col-stat layout first (critical path) ----------
        # (p t): row p*nt+t at [p, t*D:(t+1)*D] -- contiguous 2KB per partition
        x_sp_pt = const.tile([128, NT * D], F32)
        nc.sync.dma_start(x_sp_pt[:], sp.rearrange("(p t) d -> p t d", p=128))
        x_tm_pt = const.tile([128, TT * D], F32)
        nc.sync.dma_start(x_tm_pt[:], tm.rearrange("(p t) d -> p t d", p=128))
        # (t p): row t*128+p at [p, t*D:(t+1)*D] -- for row-side bias tiles
        x_sp_tp = const.tile([128, NT * D], F32)
        nc.sync.dma_start(x_sp_tp[:], sp.rearrange("(t p) d -> p t d", p=128))
        x_tm_tp = const.tile([128, TT * D], F32)
        nc.sync.dma_start(x_tm_tp[:], tm.rearrange("(t p) d -> p t d", p=128))

        # broadcast weights straight from DRAM with step-0 partition APs.
        # col-side pairs: wc_sp = [w_ss2, w_ts2], wc_tm = [w_st2, w_tt2]
        # row-side pairs: wr_sp = [w_ss1, w_st1], wr_tm = [w_ts1, w_tt1]
        def wload(name, spec):
            t = const.tile([128, 2 * D], F32, name=name, tag=name)
            for i, (nm, half) in enumerate(spec):
                src = ws[f"w_{nm}"][half * D : (half + 1) * D]
                nc.scalar.dma_start(
                    t[:, i * D : (i + 1) * D], src.unsqueeze(0).broadcast_to((128, D))
                )
            return t

        bb = const.tile([128, 4], F32)   # b_ss, b_st, b_ts, b_tt broadcast
        for j, nm in enumerate(QUADS):
            nc.scalar.dma_start(
                bb[:, j : j + 1], ws[f"b_{nm}"].unsqueeze(0).broadcast_to((128, 1))
            )
        wc_sp = wload("wc_sp", [("ss", 1), ("ts", 1)])
        wc_tm = wload("wc_tm", [("st", 1), ("tt", 1)])
        wr_sp = wload("wr_sp", [("ss", 0), ("st", 0)])
        wr_tm = wload("wr_tm", [("ts", 0), ("tt", 0)])

        # ---- stats on VectorE: mul + reduce over D ------------------------
        def stats(x, wpair, nt, name, nslots=2, bias=None, store=None):
            # per-slot mul/reduce so downstream stores fire ASAP
            st = const.tile(
                [128, nslots * nt], F32, name=f"stat_{name}", tag=f"stat_{name}"
            )
            prod = const.tile(
                [128, nt * D], F32, name=f"prod_{name}", tag=f"prod_{name}"
            )
            x3 = x[:].rearrange("p (t d) -> p t d", t=nt)
            p3 = prod[:].rearrange("p (t d) -> p t d", t=nt)
            for s in range(nslots):
                w3 = wpair[:, s * D : (s + 1) * D].unsqueeze(1).broadcast_to(
                    (128, nt, D)
                )
                nc.vector.tensor_tensor(p3, x3, w3, OP.mult)
                sl = st[:, s * nt : (s + 1) * nt]
                nc.vector.tensor_reduce(
                    sl, p3, axis=mybir.AxisListType.X, op=OP.add
                )
                if bias is not None:
                    nc.vector.tensor_scalar_add(sl, sl, bias[s])
                if store is not None:
                    nc.sync.dma_start(store[s], sl)
            return st

        # col stats: (p t) layout; emit mul/reduce/bias per slot, store the
        # slot to its scratch range, and immediately queue the broadcast-load
        # for that range so col_sp[0:N] (the first ACT dependency) is never
        # stuck behind unrelated scratch stores in the ring FIFO.
        col_sp = const.tile([128, W], F32)
        col_tm = const.tile([128, W], F32)

        def cstat_slot(x, nt, w, b, scr_rng, col_dst, name):
            prod = const.tile([128, nt * D], F32, name=f"prod_{name}", tag="cprod")
            x3 = x[:].rearrange("p (t d) -> p t d", t=nt)
            p3 = prod[:].rearrange("p (t d) -> p t d", t=nt)
            w3 = w.unsqueeze(1).broadcast_to((128, nt, D))
            nc.vector.tensor_tensor(p3, x3, w3, OP.mult)
            st = const.tile([128, nt], F32, name=f"cstat_{name}", tag=f"cstat_{name}")
            nc.vector.tensor_reduce(st[:], p3, axis=mybir.AxisListType.X, op=OP.add)
            nc.vector.tensor_scalar_add(st[:], st[:], b)
            nc.sync.dma_start(scr_rng, st[:])
            nc.sync.dma_start(
                col_dst, scr_rng.unsqueeze(0).broadcast_to((128, scr_rng.shape[0]))
            )

        cstat_slot(x_sp_pt, NT, wc_sp[:, 0:D], bb[:, 0:1],
                   scr["sp"][0:N], col_sp[:, 0:N], "ss")
        cstat_slot(x_tm_pt, TT, wc_tm[:, 0:D], bb[:, 1:2],
                   scr["sp"][N:W], col_sp[:, N:W], "st")
        cstat_slot(x_sp_pt, NT, wc_sp[:, D : 2 * D], bb[:, 2:3],
                   scr["tm"][0:N], col_tm[:, 0:N], "ts")
        cstat_slot(x_tm_pt, TT, wc_tm[:, D : 2 * D], bb[:, 3:4],
                   scr["tm"][N:W], col_tm[:, N:W], "tt")

        # row stats: (t p) layout, slots [a_ss, a_st] / [a_ts, a_tt]
        r_sp = stats(x_sp_tp, wr_sp, NT, "r_sp")
        r_tm = stats(x_tm_tp, wr_tm, TT, "r_tm")

        # ---- main loop: 20 output row-tiles of [128, 2560] ----------------
        def row_block(k, row0, col, st, nt, t):
            ot = outp.tile([128, W], F32, name=f"ot{k}", tag="ot")
            nc.scalar.activation(
                ot[:, 0:N], col[:, 0:N], AF.Tanh, bias=st[:, t : t + 1]
            )
            nc.scalar.activation(
                ot[:, N:W], col[:, N:W], AF.Tanh, bias=st[:, nt + t : nt + t + 1]
            )
            nc.vector.tensor_scalar_max(ot[:], ot[:], 0.0)
            nc.sync.dma_start(adj[row0 : row0 + 128, :], ot[:])

        for t in range(NT):
            row_block(t, 128 * t, col_sp, r_sp, NT, t)
        for t in range(TT):
            row_block(NT + t, N + 128 * t, col_tm, r_tm, TT, t)


def build_nc(num_devices=8):
    nc = bacc.Bacc(
        "TRN2",
        target_bir_lowering=False,
        debug=False,
        enable_asserts=True,
        num_devices=num_devices,
    )
    sp = nc.dram_tensor("spatial_nodes", (N, D), F32, kind="ExternalInput").ap()
    tm = nc.dram_tensor("temporal_nodes", (T, D), F32, kind="ExternalInput").ap()
    ws = {}
    for nm in QUADS:
        ws[f"w_{nm}"] = nc.dram_tensor(f"w_{nm}", (2 * D,), F32, kind="ExternalInput").ap()
        ws[f"b_{nm}"] = nc.dram_tensor(f"b_{nm}", (1,), F32, kind="ExternalInput").ap()
    scr = {
        "sp": nc.dram_tensor("scr_sp", (W,), F32, kind="Internal").ap(),
        "tm": nc.dram_tensor("scr_tm", (W,), F32, kind="Internal").ap(),
    }
    adj = nc.dram_tensor("adj", (W, W), F32, kind="ExternalOutput").ap()

    with tile.TileContext(nc) as tc:
        _emit(tc, sp, tm, ws, scr, adj)
    nc.compile()
    return nc


def make_in_maps(inputs):
    in_maps = []
    for b in range(B):
        m = {
            "spatial_nodes": np.ascontiguousarray(inputs["spatial_nodes"][b], np.float32),
            "temporal_nodes": np.ascontiguousarray(inputs["temporal_nodes"][b], np.float32),
        }
        for nm in QUADS:
            m[f"w_{nm}"] = np.ascontiguousarray(inputs[f"w_{nm}"], np.float32)
            m[f"b_{nm}"] = np.ascontiguousarray(inputs[f"b_{nm}"], np.float32)
        in_maps.append(m)
    return in_maps


_NC = {}


def run(inputs, trace=False, trace_cores=None):
    if 8 not in _NC:
        _NC[8] = build_nc(8)
    res = run_bass_kernel_spmd(
        _NC[8], make_in_maps(inputs), core_ids=list(range(B)), trace=trace,
        trace_cores=trace_cores,
    )
    out = np.stack([res.results[i]["adj"] for i in range(B)], axis=0)
    return out, res


def kernel(**inputs) -> np.ndarray:
    out, _ = run(inputs, trace=False)
    return out



# revision 5
# speedup vs baseline: 1.1263x; 1.1263x over previous
"""Trainium2 Bass kernel for the MLPConstructor2 adjacency problem.

Computes, per batch b (one NeuronCore each, 8-way data parallel over B):
    adj[i, j] = tanh(relu(x1_i @ w1 + x2_j @ w2 + b))
for the four (spatial/temporal) quadrants of a (2560, 2560) output.

The output is an outer broadcast-sum of per-row and per-column scalar
vectors, so the kernel is HBM-write bound (26.2 MB/core). Design:

- x is staged twice, in (t p) layout for the row-side stats (so each
  128-row output tile's biases land on partitions directly) and in (p t)
  layout for the col-side stats (so the stat tile streams out to a DRAM
  scratch contiguously in row order -- no transpose anywhere).
- All eight dot-product vectors are mul(+step-0-broadcast weight)/reduce
  on VectorE; quadrant biases are folded into the col-side stats.
- The column vectors are replicated across partitions with a single
  partition-step-0 DMA broadcast-load of the scratch: pure DMA broadcast.
  No PE, no PSUM (fp32 PE matmuls are 4 cycles/row and cold-clocked).
- Main loop per 128-row output tile: 2 ScalarE tanh activations
  (per-quadrant per-partition row bias), 1 VectorE relu in place, one
  1.31 MB contiguous store, alternating Sync (HWDGE) / GpSimd (SWDGE)
  rings so two DMA queues drain in parallel.
"""

import numpy as np
from contextlib import ExitStack

import concourse.bacc as bacc
import concourse.mybir as mybir
import concourse.tile as tile
from concourse.bass_utils import run_bass_kernel_spmd

B, N, T, D = 8, 2048, 512, 32
W = N + T            # 2560
NT, TT = N // 128, T // 128   # 16, 4 row-tiles
F32 = mybir.dt.float32
BF16 = mybir.dt.bfloat16
QUADS = ("ss", "st", "ts", "tt")


def _emit(tc, sp, tm, ws, scr, adj):
    nc = tc.nc
    AF = mybir.ActivationFunctionType
    OP = mybir.AluOpType
    with ExitStack() as ctx:
        const = ctx.enter_context(tc.tile_pool(name="const", bufs=1))
        outp = ctx.enter_context(tc.tile_pool(name="outp", bufs=8))

        # ---- stage inputs, col-stat layout first (critical path) ----------
        # (p t): row p*nt+t at [p, t*D:(t+1)*D] -- contiguous 2KB per partition
        x_sp_pt = const.tile([128, NT * D], F32)
        nc.sync.dma_start(x_sp_pt[:], sp.rearrange("(p t) d -> p t d", p=128))
        x_tm_pt = const.tile([128, TT * D], F32)
        nc.sync.dma_start(x_tm_pt[:], tm.rearrange("(p t) d -> p t d", p=128))
        # (t p): row t*128+p at [p, t*D:(t+1)*D] -- for row-side bias tiles
        x_sp_tp = const.tile([128, NT * D], F32)
        nc.sync.dma_start(x_sp_tp[:], sp.rearrange("(t p) d -> p t d", p=128))
        x_tm_tp = const.tile([128, TT * D], F32)
        nc.sync.dma_start(x_tm_tp[:], tm.rearrange("(t p) d -> p t d", p=128))

        # broadcast weights straight from DRAM with step-0 partition APs.
        # col-side pairs: wc_sp = [w_ss2, w_ts2], wc_tm = [w_st2, w_tt2]
        # row-side pairs: wr_sp = [w_ss1, w_st1], wr_tm = [w_ts1, w_tt1]
        def wload(name, spec):
            t = const.tile([128, 2 * D], F32, name=name, tag=name)
            for i, (nm, half) in enumerate(spec):
                src = ws[f"w_{nm}"][half * D : (half + 1) * D]
                nc.scalar.dma_start(
                    t[:, i * D : (i + 1) * D], src.unsqueeze(0).broadcast_to((128, D))
                )
            return t

        bb = const.tile([128, 4], F32)   # b_ss, b_st, b_ts, b_tt broadcast
        for j, nm in enumerate(QUADS):
            nc.scalar.dma_start(
                bb[:, j : j + 1], ws[f"b_{nm}"].unsqueeze(0).broadcast_to((128, 1))
            )
        wc_sp = wload("wc_sp", [("ss", 1), ("ts", 1)])
        wc_tm = wload("wc_tm", [("st", 1), ("tt", 1)])
        wr_sp = wload("wr_sp", [("ss", 0), ("st", 0)])
        wr_tm = wload("wr_tm", [("ts", 0), ("tt", 0)])

        # ---- stats on VectorE: mul + reduce over D ------------------------
        def stats(x, wpair, nt, name, nslots=2, bias=None, store=None):
            # per-slot mul/reduce so downstream stores fire ASAP
            st = const.tile(
                [128, nslots * nt], F32, name=f"stat_{name}", tag=f"stat_{name}"
            )
            prod = const.tile(
                [128, nt * D], F32, name=f"prod_{name}", tag=f"prod_{name}"
            )
            x3 = x[:].rearrange("p (t d) -> p t d", t=nt)
            p3 = prod[:].rearrange("p (t d) -> p t d", t=nt)
            for s in range(nslots):
                w3 = wpair[:, s * D : (s + 1) * D].unsqueeze(1).broadcast_to(
                    (128, nt, D)
                )
                nc.vector.tensor_tensor(p3, x3, w3, OP.mult)
                sl = st[:, s * nt : (s + 1) * nt]
                nc.vector.tensor_reduce(
                    sl, p3, axis=mybir.AxisListType.X, op=OP.add
                )
                if bias is not None:
                    nc.vector.tensor_scalar_add(sl, sl, bias[s])
                if store is not None:
                    nc.sync.dma_start(store[s], sl)
            return st

        # col stats: (p t) layout; emit mul/reduce/bias per slot, store the
        # slot to its scratch range, and immediately queue the broadcast-load
        # for that range so col_sp[0:N] (the first ACT dependency) is never
        # stuck behind unrelated scratch stores in the ring FIFO.
        col_sp = const.tile([128, W], F32)
        col_tm = const.tile([128, W], F32)

        def cstat_slot(x, nt, w, b, scr_rng, col_dst, name):
            prod = const.tile([128, nt * D], F32, name=f"prod_{name}", tag="cprod")
            x3 = x[:].rearrange("p (t d) -> p t d", t=nt)
            p3 = prod[:].rearrange("p (t d) -> p t d", t=nt)
            w3 = w.unsqueeze(1).broadcast_to((128, nt, D))
            nc.vector.tensor_tensor(p3, x3, w3, OP.mult)
            st = const.tile([128, nt], F32, name=f"cstat_{name}", tag=f"cstat_{name}")
            nc.vector.tensor_reduce(st[:], p3, axis=mybir.AxisListType.X, op=OP.add)
            nc.vector.tensor_scalar_add(st[:], st[:], b)
            nc.sync.dma_start(scr_rng, st[:])
            nc.sync.dma_start(
                col_dst, scr_rng.unsqueeze(0).broadcast_to((128, scr_rng.shape[0]))
            )

        cstat_slot(x_sp_pt, NT, wc_sp[:, 0:D], bb[:, 0:1],
                   scr["sp"][0:N], col_sp[:, 0:N], "ss")
        cstat_slot(x_tm_pt, TT, wc_tm[:, 0:D], bb[:, 1:2],
                   scr["sp"][N:W], col_sp[:, N:W], "st")
        cstat_slot(x_sp_pt, NT, wc_sp[:, D : 2 * D], bb[:, 2:3],
                   scr["tm"][0:N], col_tm[:, 0:N], "ts")
        cstat_slot(x_tm_pt, TT, wc_tm[:, D : 2 * D], bb[:, 3:4],
                   scr["tm"][N:W], col_tm[:, N:W], "tt")

        # row stats: (t p) layout, slots [a_ss, a_st] / [a_ts, a_tt]
        r_sp = stats(x_sp_tp, wr_sp, NT, "r_sp")
        r_tm = stats(x_tm_tp, wr_tm, TT, "r_tm")

        # ---- main loop: 20 output row-tiles of [128, 2560] ----------------
        def row_block(k, row0, col, st, nt, t):
            ot = outp.tile([128, W], BF16, name=f"ot{k}", tag="ot")
            nc.scalar.activation(
                ot[:, 0:N], col[:, 0:N], AF.Tanh, bias=st[:, t : t + 1]
            )
            nc.scalar.activation(
                ot[:, N:W], col[:, N:W], AF.Tanh, bias=st[:, nt + t : nt + t + 1]
            )
            nc.vector.tensor_scalar_max(ot[:], ot[:], 0.0)
            nc.sync.dma_start(adj[row0 : row0 + 128, :], ot[:])

        for t in range(NT):
            row_block(t, 128 * t, col_sp, r_sp, NT, t)
        for t in range(TT):
            row_block(NT + t, N + 128 * t, col_tm, r_tm, TT, t)


def build_nc(num_devices=8):
    nc = bacc.Bacc(
        "TRN2",
        target_bir_lowering=False,
        debug=False,
        enable_asserts=True,
        num_devices=num_devices,
    )
    sp = nc.dram_tensor("spatial_nodes", (N, D), F32, kind="ExternalInput").ap()
    tm = nc.dram_tensor("temporal_nodes", (T, D), F32, kind="ExternalInput").ap()
    ws = {}
    for nm in QUADS:
        ws[f"w_{nm}"] = nc.dram_tensor(f"w_{nm}", (2 * D,), F32, kind="ExternalInput").ap()
        ws[f"b_{nm}"] = nc.dram_tensor(f"b_{nm}", (1,), F32, kind="ExternalInput").ap()
    scr = {
        "sp": nc.dram_tensor("scr_sp", (W,), F32, kind="Internal").ap(),
        "tm": nc.dram_tensor("scr_tm", (W,), F32, kind="Internal").ap(),
    }
    adj = nc.dram_tensor("adj", (W, W), BF16, kind="ExternalOutput").ap()

    with tile.TileContext(nc) as tc:
        _emit(tc, sp, tm, ws, scr, adj)
    nc.compile()
    return nc


def make_in_maps(inputs):
    in_maps = []
    for b in range(B):
        m = {
            "spatial_nodes": np.ascontiguousarray(inputs["spatial_nodes"][b], np.float32),
            "temporal_nodes": np.ascontiguousarray(inputs["temporal_nodes"][b], np.float32),
        }
        for nm in QUADS:
            m[f"w_{nm}"] = np.ascontiguousarray(inputs[f"w_{nm}"], np.float32)
            m[f"b_{nm}"] = np.ascontiguousarray(inputs[f"b_{nm}"], np.float32)
        in_maps.append(m)
    return in_maps


_NC = {}


def run(inputs, trace=False, trace_cores=None):
    if 8 not in _NC:
        _NC[8] = build_nc(8)
    res = run_bass_kernel_spmd(
        _NC[8], make_in_maps(inputs), core_ids=list(range(B)), trace=trace,
        trace_cores=trace_cores,
    )
    out = np.stack(
        [np.asarray(res.results[i]["adj"]).astype(np.float32) for i in range(B)],
        axis=0,
    )
    return out, res


def kernel(**inputs) -> np.ndarray:
    out, _ = run(inputs, trace=False)
    return out

